# revision 8
# baseline (speedup 1.0000x reference)
"""Trainium2 Bass kernel: additive (Bahdanau-style) attention layer.

reference:
    wf    = features @ Wk + Wb            # [B, T, U]
    uh    = hidden @ Uk + Ub              # [B, 1, U]
    score = tanh(wf + uh)                 # [B, T, U]
    logit = score @ Vk + Vb               # [B, T, 1]
    attn  = softmax(logit, axis=T)
    ctx   = sum_T(attn * features)        # [B, D]
    returns (ctx, attn)

Strategy: pure data-parallel over batch (32 -> 4 per core, 8 cores, no
collectives).  Per core, everything is computed in "layout B" where the
unit axis U lives on SBUF partitions:

    fT[d, t]  = transpose(features)      via TensorE transposes (bf16)
    wfT[u, t] = sum_d Wk[d,u] * fT[d,t]  TensorE, PSUM accumulate
    scoreT    = tanh(wfT + bias[u])      ScalarE, bias = (uh+Ub+Wb)^T per
                                         partition, fused PSUM->SBUF
    logit[t]  = sum_u Vk[u]*scoreT[u,t]  TensorE (M=1)
    softmax over T in [4, 2048] layout   VectorE/ScalarE
    ctx[d]    = sum_t attn[t]*f[t,d]     TensorE (lhsT = attn column)

Vb is mathematically irrelevant (softmax shift invariance) and ignored.
Compute dtype bf16 (cast during DMA), accumulation fp32.
"""

import numpy as np

B, T, D, H, U = 32, 2048, 512, 512, 512
N_CORES = 8
BPC = B // N_CORES      # batches per core
P = 128                 # partitions
NG = 4                  # token groups per batch
GT = T // NG            # tokens per group (512)
NT = GT // P            # token tiles per group (4)
DB = D // P             # d blocks (4)
UB = U // P             # u blocks (4)
HB = H // P             # h blocks (4)
TPB = T // P            # token tiles per batch (16)

_CACHE = {}


def _build():
    import concourse.bacc as bacc
    import concourse.tile as tile
    from concourse import mybir
    from concourse.masks import make_identity

    f32 = mybir.dt.float32
    bf16 = mybir.dt.bfloat16
    AF = mybir.ActivationFunctionType

    nc = bacc.Bacc(
        "TRN2",
        target_bir_lowering=False,
        debug=False,
        num_devices=N_CORES,
    )

    feat = nc.declare_dram_parameter("features", [BPC * T, D], f32, isOutput=False).ap()
    hid = nc.declare_dram_parameter("hidden", [BPC, H], f32, isOutput=False).ap()
    wk_d = nc.declare_dram_parameter("Wk", [D, U], f32, isOutput=False).ap()
    wb_d = nc.declare_dram_parameter("Wb", [U], f32, isOutput=False).ap()
    uk_d = nc.declare_dram_parameter("Uk", [H, U], f32, isOutput=False).ap()
    ub_d = nc.declare_dram_parameter("Ub", [U], f32, isOutput=False).ap()
    vk_d = nc.declare_dram_parameter("Vk", [U, 1], f32, isOutput=False).ap()
    ctx_out = nc.declare_dram_parameter("ctx", [BPC, D], f32, isOutput=True).ap()
    attn_out = nc.declare_dram_parameter("attn", [BPC, T], f32, isOutput=True).ap()

    with tile.TileContext(nc) as tc:
        with (
            tc.tile_pool(name="const", bufs=1) as cpool,
            tc.tile_pool(name="ftp", bufs=1) as ftpool,
            tc.tile_pool(name="fTp", bufs=2) as fTpool,
            tc.tile_pool(name="scp", bufs=2) as scpool,
            tc.tile_pool(name="ps_fT", bufs=2, space="PSUM") as ps_fT,
            tc.tile_pool(name="ps_wfT", bufs=2, space="PSUM") as ps_wfT,
            tc.tile_pool(name="ps_mm1", bufs=2, space="PSUM") as ps_mm1,
            tc.tile_pool(name="ps_aT", bufs=1, space="PSUM") as ps_aT,
            tc.tile_pool(name="ps_pre", bufs=1, space="PSUM") as ps_pre,
        ):
            # ---------- constants ----------
            ident32 = cpool.tile([P, P], f32)
            make_identity(nc, ident32[:])
            ident16 = cpool.tile([P, P], bf16)
            make_identity(nc, ident16[:])

            wk_sb = []
            for j in range(DB):
                t = cpool.tile([P, U], bf16, tag=f"wk{j}")
                nc.gpsimd.dma_start(t[:], wk_d[j * P:(j + 1) * P, :])  # f32->bf16
                wk_sb.append(t)
            uk_sb = []
            for j in range(HB):
                t = cpool.tile([P, U], f32, tag=f"uk{j}")
                nc.sync.dma_start(t[:], uk_d[j * P:(j + 1) * P, :])
                uk_sb.append(t)
            vkT = cpool.tile([P, UB], bf16)
            nc.gpsimd.dma_start(vkT[:], vk_d.rearrange("(b p) o -> p (b o)", p=P))
            wbT = cpool.tile([P, UB], f32)
            nc.sync.dma_start(wbT[:], wb_d.rearrange("(b p) -> p b", p=P))
            ubT = cpool.tile([P, UB], f32)
            nc.sync.dma_start(ubT[:], ub_d.rearrange("(b p) -> p b", p=P))
            wubT = cpool.tile([P, UB], f32)
            nc.vector.tensor_add(wubT[:], wbT[:], ubT[:])

            # ---------- features: load all (resident), f32 -> bf16 in DMA ----------
            ft = {}
            for b in range(BPC):
                for g in range(NG):
                    t = ftpool.tile([P, NT, D], bf16, tag=f"ft_{b}_{g}")
                    src = feat[b * T + g * GT: b * T + (g + 1) * GT, :]
                    nc.gpsimd.dma_start(t[:], src.rearrange("(i p) d -> p i d", p=P))
                    ft[(b, g)] = t

            # ---------- uh = hidden @ Uk ; bias = (uh + Ub + Wb)^T ----------
            hid_sb = cpool.tile([BPC, H], f32)
            nc.sync.dma_start(hid_sb[:], hid[:, :])
            hT_ps = ps_pre.tile([P, HB * BPC], f32, tag="pre")
            for j in range(HB):
                nc.tensor.transpose(
                    hT_ps[:, j * BPC:(j + 1) * BPC],
                    hid_sb[:, j * P:(j + 1) * P],
                    ident32[:BPC, :BPC],
                )
            hT_sb = cpool.tile([P, HB * BPC], f32)
            nc.vector.tensor_copy(hT_sb[:], hT_ps[:])

            uh_ps = ps_pre.tile([BPC, U], f32, tag="pre")
            for j in range(HB):
                nc.tensor.matmul(
                    uh_ps[:],
                    hT_sb[:, j * BPC:(j + 1) * BPC],
                    uk_sb[j][:],
                    start=(j == 0),
                    stop=(j == HB - 1),
                )
            uh_sb = cpool.tile([BPC, U], f32)
            nc.vector.tensor_copy(uh_sb[:], uh_ps[:])

            uhT_ps = ps_pre.tile([P, UB * BPC], f32, tag="pre")
            for jb in range(UB):
                nc.tensor.transpose(
                    uhT_ps[:, jb * BPC:(jb + 1) * BPC],
                    uh_sb[:, jb * P:(jb + 1) * P],
                    ident32[:BPC, :BPC],
                )
            bias_sb = cpool.tile([P, UB * BPC], f32)
            for jb in range(UB):
                nc.vector.tensor_scalar_add(
                    bias_sb[:, jb * BPC:(jb + 1) * BPC],
                    uhT_ps[:, jb * BPC:(jb + 1) * BPC],
                    wubT[:, jb:jb + 1],
                )

            # batch b's logits live at partition 32*b (legal engine base partitions)
            logits_sb = cpool.tile([P, T], f32)
            nc.gpsimd.memset(logits_sb[:], 0.0)

            # ---------- main pipeline over 16 groups, 2-deep skew ----------
            groups = [(b, g) for b in range(BPC) for g in range(NG)]
            fT_of = {}
            sc_of = {}

            def stage_T(idx):
                b, g = groups[idx]
                fgrp = ft[(b, g)]
                tiles = []
                for j in range(DB):
                    fT_ps = ps_fT.tile([P, GT], bf16, tag="fTps")
                    for i in range(NT):
                        nc.tensor.transpose(
                            fT_ps[:, i * P:(i + 1) * P],
                            fgrp[:, i, j * P:(j + 1) * P],
                            ident16[:, :],
                        )
                    t = fTpool.tile([P, GT], bf16, tag=f"fT{j}")
                    nc.vector.tensor_copy(t[:], fT_ps[:])
                    tiles.append(t)
                fT_of[idx] = tiles

            def stage_MM(idx):
                b, g = groups[idx]
                tiles = fT_of.pop(idx)
                scs = []
                for ub_i in range(UB):
                    wfT_ps = ps_wfT.tile([P, GT], f32, tag="wfT")
                    for j in range(DB):
                        nc.tensor.matmul(
                            wfT_ps[:],
                            wk_sb[j][:, ub_i * P:(ub_i + 1) * P],
                            tiles[j][:],
                            start=(j == 0),
                            stop=(j == DB - 1),
                        )
                    sc = scpool.tile([P, GT], bf16, tag=f"sc{ub_i}")
                    nc.scalar.activation(
                        sc[:],
                        wfT_ps[:],
                        AF.Tanh,
                        bias=bias_sb[:, ub_i * BPC + b: ub_i * BPC + b + 1],
                        scale=1.0,
                    )
                    scs.append(sc)
                sc_of[idx] = scs

            def stage_LG(idx):
                b, g = groups[idx]
                scs = sc_of.pop(idx)
                lg_ps = ps_mm1.tile([1, GT], f32, tag="mm1")
                for ub_i in range(UB):
                    nc.tensor.matmul(
                        lg_ps[:],
                        vkT[:, ub_i:ub_i + 1],
                        scs[ub_i][:],
                        start=(ub_i == 0),
                        stop=(ub_i == UB - 1),
                    )
                nc.scalar.copy(logits_sb[32 * b:32 * b + 1, g * GT:(g + 1) * GT], lg_ps[:])

            n = len(groups)
            for k in range(n + 2):
                if k < n:
                    stage_T(k)
                if 1 <= k <= n:
                    stage_MM(k - 1)
                if 2 <= k:
                    stage_LG(k - 2)

            # ---------- softmax over T, all 4 batches at once ----------
            # rows 32b hold batch b; other partitions hold zeros (harmless)
            neg_mx = cpool.tile([P, 1], f32)
            nc.vector.reduce_max(
                neg_mx[:], logits_sb[:], axis=mybir.AxisListType.X, negate=True
            )
            e_sb = cpool.tile([P, T], f32)
            nc.scalar.activation(e_sb[:], logits_sb[:], AF.Exp, bias=neg_mx[:], scale=1.0)
            s_sb = cpool.tile([P, 1], f32)
            nc.vector.reduce_sum(s_sb[:], e_sb[:], axis=mybir.AxisListType.X)
            r_sb = cpool.tile([P, 1], f32)
            nc.vector.reciprocal(r_sb[:], s_sb[:])
            attn_sb = cpool.tile([P, T], f32)
            nc.vector.tensor_scalar_mul(attn_sb[:], e_sb[:], r_sb[:])
            nc.sync.dma_start(attn_out[:, :], attn_sb[0:P:32, :])

            # ---------- context = attn^T-weighted sum of features ----------
            aT_sb = cpool.tile([P, TPB * BPC], bf16)  # col = i*BPC + b
            aT_ps = ps_aT.tile([P, TPB * BPC], f32, tag="aT")
            for b in range(BPC):
                for i in range(TPB):
                    nc.tensor.transpose(
                        aT_ps[:, i * BPC + b: i * BPC + b + 1],
                        attn_sb[32 * b:32 * b + 1, i * P:(i + 1) * P],
                        ident32[32 * b:32 * b + 1, 32 * b:32 * b + 1],
                        tile_position=(32 * b, 0),
                    )
            nc.vector.tensor_copy(aT_sb[:], aT_ps[:])

            ctx_sb = cpool.tile([P, D], f32)
            for b in range(BPC):
                cx_ps = ps_mm1.tile([1, D], f32, tag="mm1")
                for i in range(TPB):
                    g, ii = divmod(i, NT)
                    nc.tensor.matmul(
                        cx_ps[:],
                        aT_sb[:, i * BPC + b: i * BPC + b + 1],
                        ft[(b, g)][:, ii, :],
                        start=(i == 0),
                        stop=(i == TPB - 1),
                    )
                nc.scalar.copy(ctx_sb[32 * b:32 * b + 1, :], cx_ps[:])
            nc.sync.dma_start(ctx_out[:, :], ctx_sb[0:P:32, :])

    nc.compile()
    return nc


def _get_nc():
    if "nc" not in _CACHE:
        _CACHE["nc"] = _build()
    return _CACHE["nc"]


def _shard(features, hidden, Wk, Wb, Uk, Ub, Vk):
    f32 = np.float32
    features = np.asarray(features, dtype=f32).reshape(B, T, D)
    hidden = np.asarray(hidden, dtype=f32)
    Wk = np.ascontiguousarray(np.asarray(Wk, dtype=f32))
    Wb = np.ascontiguousarray(np.asarray(Wb, dtype=f32))
    Uk = np.ascontiguousarray(np.asarray(Uk, dtype=f32))
    Ub = np.ascontiguousarray(np.asarray(Ub, dtype=f32))
    Vk = np.ascontiguousarray(np.asarray(Vk, dtype=f32).reshape(U, 1))
    in_maps = []
    for c in range(N_CORES):
        sl = slice(c * BPC, (c + 1) * BPC)
        in_maps.append({
            "features": np.ascontiguousarray(features[sl]).reshape(BPC * T, D),
            "hidden": np.ascontiguousarray(hidden[sl]),
            "Wk": Wk, "Wb": Wb, "Uk": Uk, "Ub": Ub, "Vk": Vk,
        })
    return in_maps


def _run(in_maps, trace=False, tmpdir=None):
    from concourse.bass_utils import run_bass_kernel_spmd
    nc = _get_nc()
    return run_bass_kernel_spmd(
        nc, in_maps, core_ids=list(range(N_CORES)), trace=trace, tmpdir=tmpdir
    )


def kernel(features, hidden, Wk, Wb, Uk, Ub, Vk, Vb=None, **_ignored):
    in_maps = _shard(features, hidden, Wk, Wb, Uk, Ub, Vk)
    res = _run(in_maps)
    ctx = np.concatenate([r["ctx"] for r in res.results], axis=0)
    attn = np.concatenate([r["attn"] for r in res.results], axis=0)
    return ctx.astype(np.float32), attn.reshape(B, T, 1).astype(np.float32)
